# revision 32
# baseline (speedup 1.0000x reference)
"""Multi-head attention (12 heads, N=4096, C=768) on 8 TRN2 NeuronCores.

Sharding: 8 cores = 4 head-groups x 2 sequence halves.
  core c: heads 3*(c%4) .. 3*(c%4)+2, query rows half (c//4).
Each core computes K/V projections for its 3 heads over the FULL sequence
(inputs are passed with the core's query half rotated to the front, which is
legal because softmax+PV is permutation-invariant along the key axis), Q for
its 2048 query rows, eager attention in S^T orientation (keys on PSUM
partitions), and a partial output projection. Host sums the 4 head-group
partials per sequence half and adds the bias terms.

Performance structure (cost model: matmul time ~ moving free size only):
 - PV runs with exp(S) tiles as the stationary operand ([128k x 128q]) and
   [V | 1] as the 65-wide moving operand, so each accumulation step costs 65
   rows instead of 512. PV accumulates with start=False onto memset-zeroed
   PSUM banks (several accumulation windows share a bank, so the start=True
   bank-wide zero marking cannot be used).
 - Head-2 K and Q projections share one stationary matrix (128 output
   partitions); the missing bq2 is folded into the softmax as a
   per-partition activation bias K.bq2 (constant along q in S^T layout).
 - Q/bq are pre-scaled by 16*log2(e) on the host so the same S feeds both
   the exact-exp path (scale ln2/128) and a future bit-trick exp path.
 - Output projection uses transposed attention tiles as the stationary
   operand: 2 hd-chunks x (512+256) moving columns per 128-query tile.
 - x^T is DMA'd in 8 sequence slabs and the first attention unit is fused
   into the projection loop, so the Act engine (the bottleneck: 192 exp
   instructions) starts within a few microseconds.

Bias algebra (exact): bk drops out of softmax entirely; bv contributes
bv @ Wo to every output row (added on host with bo); bq is folded into Q
(heads 0,1) or into the activation bias (head 2).

All matmuls run with bf16 inputs and fp32 PSUM accumulation.
"""

import math

import numpy as np
import ml_dtypes

import concourse.bass as bass
from concourse import bacc
import concourse.tile as tile
import concourse.mybir as mybir
from concourse.bass_utils import run_bass_kernel_spmd

P = 128
C = 768                    # hidden
NSEQ = 4096                # sequence length
HPC = 3                    # heads per core
HD = 64                    # head dim
HW = HPC * HD              # 192, projection width per core
QB = 2048                  # query rows per core
QBLK = 1024                # query block (PSUM-friendly)
NCH = C // P               # 6 contraction chunks
KT = NSEQ // P             # 32 key tiles
BF16 = mybir.dt.bfloat16
F32 = mybir.dt.float32
AF = mybir.ActivationFunctionType
ALU = mybir.AluOpType

PRE = 16.0 * math.log2(math.e)     # host-side Q pre-scale
SCL = math.log(2.0) / 128.0        # activation scale: exp(S_pre*SCL)=exp(S*0.125)
MAGIC = 16256.0 - 7.5              # bf16 Schraudolph: int16(S_pre + MAGIC) ~ 2^(S_pre/128)
                                   # (-7.5 centers the multiplicative error so
                                   # approximated key-tiles are unbiased vs the
                                   # exact-exp tiles they mix with in softmax)

_CACHE = {}

# set by test.py to capture profiling info
TRACE = False
LAST_RESULT = None


def _build():
    nc = bacc.Bacc("TRN2")

    xT = nc.dram_tensor("xT", [C, NSEQ], BF16, kind="ExternalInput")
    wq01 = nc.dram_tensor("wq01", [C, P], BF16, kind="ExternalInput")
    wk01 = nc.dram_tensor("wk01", [C, P], BF16, kind="ExternalInput")
    wkq2 = nc.dram_tensor("wkq2", [C, P], BF16, kind="ExternalInput")
    wv = nc.dram_tensor("wv", [C, HW], BF16, kind="ExternalInput")
    woA = nc.dram_tensor("woA", [P, C], BF16, kind="ExternalInput")
    woB = nc.dram_tensor("woB", [HD, C], BF16, kind="ExternalInput")
    bq01 = nc.dram_tensor("bq01", [1, P], BF16, kind="ExternalInput")
    bq2c = nc.dram_tensor("bq2c", [HD, 1], BF16, kind="ExternalInput")
    ident = nc.dram_tensor("ident", [P, P], BF16, kind="ExternalInput")
    out = nc.dram_tensor("out", [QB, C], BF16, kind="ExternalOutput")

    NSLAB = 8
    SLAB = NSEQ // NSLAB  # 512

    with tile.TileContext(nc) as tc:
        with (
            tc.tile_pool(name="const", bufs=1) as const,
            tc.tile_pool(name="proj", bufs=1) as proj,
            tc.tile_pool(name="pt", bufs=8) as ptp,
            tc.tile_pool(name="stage", bufs=2) as stage,
            tc.tile_pool(name="psS", bufs=2, space="PSUM") as psS,
            tc.tile_pool(name="psO", bufs=1, space="PSUM") as psO,
            tc.tile_pool(name="psX", bufs=2, space="PSUM") as psX,
        ):
            # ---- load inputs; ordered so Q/K projections can start ASAP ----
            xt = const.tile([P, NCH, NSEQ], BF16)

            def slab_dma(sl):
                nc.sync.dma_start(
                    xt[:, :, sl * SLAB : (sl + 1) * SLAB],
                    xT[:, sl * SLAB : (sl + 1) * SLAB].rearrange(
                        "(c p) n -> p c n", p=P
                    ),
                )

            slab_dma(0)
            wq01_sb = const.tile([P, NCH, P], BF16)
            nc.sync.dma_start(wq01_sb[:], wq01[:].rearrange("(c p) m -> p c m", p=P))
            bq01_sb = const.tile([1, P], BF16)
            nc.sync.dma_start(bq01_sb[:], bq01[:])
            slab_dma(1)
            wk01_sb = const.tile([P, NCH, P], BF16)
            nc.sync.dma_start(wk01_sb[:], wk01[:].rearrange("(c p) m -> p c m", p=P))
            wkq2_sb = const.tile([P, NCH, P], BF16)
            nc.sync.dma_start(wkq2_sb[:], wkq2[:].rearrange("(c p) m -> p c m", p=P))
            wv_sb = const.tile([P, NCH, HW], BF16)
            nc.sync.dma_start(wv_sb[:], wv[:].rearrange("(c p) m -> p c m", p=P))
            for sl in range(2, NSLAB):
                slab_dma(sl)
            ident_sb = const.tile([P, P], BF16)
            nc.sync.dma_start(ident_sb[:], ident[:])
            bq2_sb = const.tile([HD, 1], BF16)
            nc.sync.dma_start(bq2_sb[:], bq2c[:])
            woA_sb = const.tile([P, C], BF16)
            nc.sync.dma_start(woA_sb[:], woA[:])
            woB_sb = const.tile([HD, C], BF16)
            nc.sync.dma_start(woB_sb[:], woB[:])

            ones_row = const.tile([1, 512], BF16)
            nc.vector.memset(ones_row[:], 1.0)

            # ---- persistent projection outputs ----
            KT01 = proj.tile([P, NSEQ], BF16)     # rows 0:64 h0 K^T, 64:128 h1 K^T
            KQ2 = proj.tile([P, NSEQ], BF16)      # rows 0:64 h2 K^T, 64:128 h2 Q^T
            QT01 = proj.tile([P, QB], BF16)       # rows 0:64 h0 Q^T, 64:128 h1 Q^T
            QT2 = proj.tile([HD, QB], BF16)       # h2 Q^T at partition base 0
            V_sb = proj.tile([P, KT, HPC, HD + 1], BF16)  # V + ones column
            O_sb = proj.tile([P, 8, HPC, HD], BF16)       # per-qb normalized attn
            attnTA = proj.tile([P, QB], BF16)     # attn^T rows: h0 d + h1 d
            attnTB = proj.tile([HD, QB], BF16)    # attn^T rows: h2 d
            bias_ln = proj.tile([P, KT], F32)     # (K2.bq2)*0.125 per kt
            bias_pre = proj.tile([P, KT], F32)    # (K2.bq2)*PRE per kt

            nc.vector.memset(V_sb[:, :, :, HD : HD + 1], 1.0)

            # ---------- building blocks ----------
            def q01_proj(nt):
                ps = psX.tile([P, 512], F32, tag="ps")
                for c in range(NCH):
                    nc.tensor.matmul(
                        ps[:], wq01_sb[:, c, :], xt[:, c, nt * 512 : (nt + 1) * 512],
                        start=(c == 0), stop=False,
                    )
                nc.tensor.matmul(
                    ps[:], bq01_sb[0:1, :], ones_row[0:1, :], start=False, stop=True
                )
                nc.vector.tensor_copy(QT01[:, nt * 512 : (nt + 1) * 512], ps[:])

            def k01_proj(nt):
                ps = psX.tile([P, 512], F32, tag="ps")
                for c in range(NCH):
                    nc.tensor.matmul(
                        ps[:], wk01_sb[:, c, :], xt[:, c, nt * 512 : (nt + 1) * 512],
                        start=(c == 0), stop=(c == NCH - 1),
                    )
                nc.vector.tensor_copy(KT01[:, nt * 512 : (nt + 1) * 512], ps[:])

            def kq2_proj(nt):
                ps = psX.tile([P, 512], F32, tag="ps")
                for c in range(NCH):
                    nc.tensor.matmul(
                        ps[:], wkq2_sb[:, c, :], xt[:, c, nt * 512 : (nt + 1) * 512],
                        start=(c == 0), stop=(c == NCH - 1),
                    )
                nc.vector.tensor_copy(KQ2[:, nt * 512 : (nt + 1) * 512], ps[:])
                if nt < QB // 512:
                    nc.vector.tensor_copy(
                        QT2[:, nt * 512 : (nt + 1) * 512], ps[HD:P, :]
                    )

            def v_proj(kt):
                ps = psX.tile([P, 512], F32, tag="ps")
                for c in range(NCH):
                    nc.tensor.matmul(
                        ps[:, 0:HW], xt[:, c, kt * P : (kt + 1) * P], wv_sb[:, c, :],
                        start=(c == 0), stop=(c == NCH - 1),
                    )
                nc.vector.tensor_copy(
                    V_sb[:, kt, :, 0:HD],
                    ps[:, 0:HW].rearrange("p (h d) -> p h d", d=HD),
                )

            def kt_ap(h, kt):
                if h == 0:
                    return KT01[0:HD, kt * P : (kt + 1) * P]
                if h == 1:
                    return KT01[HD:P, kt * P : (kt + 1) * P]
                return KQ2[0:HD, kt * P : (kt + 1) * P]

            def qt_ap(h, qb, qt):
                lo = qb * QBLK + qt * 512
                if h == 0:
                    return QT01[0:HD, lo : lo + 512]
                if h == 1:
                    return QT01[HD:P, lo : lo + 512]
                return QT2[:, lo : lo + 512]

            # attention unit state: two PSUM accumulators (qs 0..3 / 4..7)
            def unit_open():
                oA = psO.tile([P, 4, HD + 1], F32, tag="oA")
                oB = psO.tile([P, 4, HD + 1], F32, tag="oB")
                nc.vector.memset(oA[:], 0.0)
                nc.vector.memset(oB[:], 0.0)
                return oA, oB

            def emit_S(h, qb, kt, eng="A"):
                S_t = psS.tile([P, QBLK], F32, tag="s")
                for qt in range(2):
                    nc.tensor.matmul(
                        S_t[:, qt * 512 : (qt + 1) * 512],
                        kt_ap(h, kt), qt_ap(h, qb, qt),
                        start=True, stop=True,
                    )
                pt = ptp.tile([P, QBLK], BF16, tag="pt")
                if eng == "A":
                    bias = bias_ln[:, kt : kt + 1] if h == 2 else 0.0
                    nc.scalar.activation(pt[:], S_t[:], AF.Exp, bias=bias, scale=SCL)
                else:
                    # bit-trick exp2 on DVE: bf16 bits of 2^(S_pre/128) are
                    # int16(S_pre + MAGIC) (S is pre-scaled by 16*log2 e)
                    pt_i = pt[:].bitcast(mybir.dt.int16)
                    if h == 2:
                        nc.vector.tensor_scalar(
                            pt_i, S_t[:], bias_pre[:, kt : kt + 1], MAGIC,
                            ALU.add, ALU.add,
                        )
                    else:
                        nc.vector.tensor_scalar(pt_i, S_t[:], MAGIC, None, ALU.add)
                return pt

            def emit_PV(h, kt, pt, oA, oB):
                for qs in range(8):
                    o = oA if qs < 4 else oB
                    nc.tensor.matmul(
                        o[:, qs % 4, :],
                        pt[:, qs * P : (qs + 1) * P],
                        V_sb[:, kt, h, :],
                        start=False, stop=False, skip_group_check=True,
                    )

            def unit_close(h, oA, oB):
                # normalize: out = O[:, :64] * (1/den), den in column 64
                recA = stage.tile([P, 4, 1], F32, tag="rec")
                nc.vector.reciprocal(recA[:], oA[:, :, HD : HD + 1])
                recB = stage.tile([P, 4, 1], F32, tag="rec")
                nc.vector.reciprocal(recB[:], oB[:, :, HD : HD + 1])
                for qs in range(8):
                    o, r = (oA, recA) if qs < 4 else (oB, recB)
                    nc.vector.tensor_scalar(
                        O_sb[:, qs, h, :], o[:, qs % 4, 0:HD],
                        r[:, qs % 4, :], None, ALU.mult,
                    )

            def transpose_chunk(qb, qs, pool, tag):
                psT = pool.tile([P, 2 * P], BF16, tag=tag, name="psT")
                nc.tensor.transpose(psT[:, 0:P], O_sb[:, qs, 0:2, :], ident_sb[:])
                nc.tensor.transpose(
                    psT[0:HD, P : 2 * P], O_sb[:, qs, 2, :], ident_sb[:]
                )
                lo = qb * QBLK + qs * P
                nc.vector.tensor_copy(attnTA[:, lo : lo + P], psT[:, 0:P])
                nc.vector.tensor_copy(attnTB[:, lo : lo + P], psT[0:HD, P : 2 * P])

            def outproj_chunk(qb, qs, st, j):
                lo = qb * QBLK + qs * P
                for s0, sw in ((0, 512), (512, 256)):
                    pso = psX.tile([P, 512], F32, tag="ps")
                    nc.tensor.matmul(
                        pso[:, 0:sw], attnTA[:, lo : lo + P],
                        woA_sb[:, s0 : s0 + sw],
                        start=True, stop=False,
                    )
                    nc.tensor.matmul(
                        pso[:, 0:sw], attnTB[0:HD, lo : lo + P],
                        woB_sb[:, s0 : s0 + sw],
                        start=False, stop=True,
                    )
                    nc.vector.tensor_copy(st[:, j, s0 : s0 + sw], pso[:, 0:sw])

            def qb_finish_chunk(qb, qs):
                # transpose one 128-query tile of normalized attn into attn^T,
                # then output-project and DMA it out
                transpose_chunk(qb, qs, psX, "ps")
                st = stage.tile([P, 1, C], BF16, tag="st")
                outproj_chunk(qb, qs, st, 0)
                lo = qb * QBLK + qs * P
                nc.sync.dma_start(out[lo : lo + P, :], st[:, 0, :])

            def qb_finish_tail(qb):
                # pipelined epilogue for the final query block: all transposes
                # first (PSUM accumulators are free by now), then paired
                # output-projection chunks sharing one DMA each
                for qs in range(8):
                    transpose_chunk(qb, qs, psO, "oA" if qs % 2 == 0 else "oB")
                for pair in range(4):
                    st = stage.tile([P, 2, C], BF16, tag="st2")
                    for j in (0, 1):
                        outproj_chunk(qb, 2 * pair + j, st, j)
                    lo = qb * QBLK + pair * 2 * P
                    nc.sync.dma_start(
                        out[lo : lo + 2 * P, :].rearrange("(j p) c -> p j c", p=P),
                        st[:],
                    )

            def kbq2_bias():
                psB = psX.tile([P, KT], F32, tag="ps")
                for kt in range(KT):
                    nc.tensor.matmul(
                        psB[:, kt : kt + 1], kt_ap(2, kt), bq2_sb[:],
                        start=True, stop=True,
                    )
                nc.vector.tensor_scalar(bias_ln[:], psB[:], 0.125, None, ALU.mult)
                nc.vector.tensor_scalar(bias_pre[:], psB[:], PRE, None, ALU.mult)

            # ---------- schedule ----------
            # One flat software-pipelined stream over 192 (unit, kt) steps.
            # Step i emits S+exp for step i, then PV for step i-2 (the lag
            # gives the DVE time to close/zero PSUM accumulators at unit
            # boundaries without stalling the PE pipeline). Background work
            # (leftover projections, h2 bias, qb0 epilogue) is interleaved
            # one small chunk every other step to stay under the Act step
            # budget (~1038ns).
            LAG = 4
            NST = 6 * KT
            UNITS = [(h, qb) for qb in (0, 1) for h in (0, 1, 2)]

            bg = []
            bg.append(lambda: q01_proj(2))
            bg.append(lambda: q01_proj(3))
            bg.append(kbq2_bias)
            for qs_ in range(8):
                bg.append(lambda qs=qs_: qb_finish_chunk(0, qs))

            pts = {}   # step -> pt tile
            os_ = {}   # unit -> (oA, oB)

            def emit_PV_step(j):
                h, qb = UNITS[j // KT]
                kt = j % KT
                if kt == 0:
                    os_[j // KT] = unit_open()
                emit_PV(h, kt, pts.pop(j), *os_[j // KT])
                if kt == KT - 1:
                    unit_close(h, *os_.pop(j // KT))

            # per-step exp engine: Act / DVE balanced by cost-model throughput
            PROPAT = "AD"      # prologue: PE-bound, DVE has slack
            STEADYPAT = "AADAAADAAD"

            def exp_eng(i):
                return PROPAT[i % 2] if i < KT else STEADYPAT[i % 10]

            def stream_step(i, xa=None):
                h, qb = UNITS[i // KT]
                pts[i] = emit_S(h, qb, i % KT, exp_eng(i))
                if i >= LAG:
                    emit_PV_step(i - LAG)
                # background chunk (only after the fused projection prologue;
                # qb0 epilogue chunks additionally wait for (qb0,h2) close)
                if i >= KT and (i % 3 == 0) and bg:
                    if not (len(bg) <= 8 and i < 3 * KT + LAG + 2):
                        bg.pop(0)()

            # fused prologue: projections + unit 0, pipelined.  S/exp for a
            # block go out right after its K tile so the Act engine starts
            # early; V lags behind (PV only needs it LAG steps later).
            q01_proj(0)
            q01_proj(1)
            for nt in range(NSLAB):
                k01_proj(nt)
                for kt in range(4 * nt, 4 * nt + 4):
                    stream_step(kt)
                kq2_proj(nt)
                for kt in range(4 * nt, 4 * nt + 4):
                    v_proj(kt)
            for i in range(KT, NST):
                stream_step(i)
            # drain: last PVs + closes + leftovers + qb1 epilogue
            for j in range(NST - LAG, NST):
                emit_PV_step(j)
            while bg:
                bg.pop(0)()
            qb_finish_tail(1)

    if hasattr(nc, "compile"):
        nc.compile()
    return nc


def _get_nc():
    if "nc" not in _CACHE:
        _CACHE["nc"] = _build()
    return _CACHE["nc"]


def kernel(x, Wq, bq, Wk, bk, Wv, bv, Wo, bo):
    global LAST_RESULT
    x = np.asarray(x, dtype=np.float32)
    Wq = np.asarray(Wq, dtype=np.float32)
    Wk = np.asarray(Wk, dtype=np.float32)
    Wv = np.asarray(Wv, dtype=np.float32)
    Wo = np.asarray(Wo, dtype=np.float32)
    bq = np.asarray(bq, dtype=np.float32)
    bv = np.asarray(bv, dtype=np.float32)
    bo = np.asarray(bo, dtype=np.float32)

    B, N, Ch = x.shape
    assert (B, N, Ch) == (1, NSEQ, C)
    xT_full = np.ascontiguousarray(x[0].T)  # [C, N] f32

    bf = ml_dtypes.bfloat16
    ident = np.eye(P, dtype=np.float32)
    in_maps = []
    for c in range(8):
        qhalf = c // 4
        hbase = HPC * (c % 4)
        cols = slice(hbase * HD, hbase * HD + HW)
        c01 = slice(hbase * HD, hbase * HD + 2 * HD)
        c2 = slice(hbase * HD + 2 * HD, hbase * HD + HW)
        if qhalf == 0:
            xTc = xT_full
        else:
            xTc = np.concatenate([xT_full[:, QB:], xT_full[:, :QB]], axis=1)
        wkq2 = np.concatenate([Wk[:, c2], Wq[:, c2] * PRE], axis=1)
        in_maps.append({
            "xT": np.ascontiguousarray(xTc).astype(bf),
            "wq01": np.ascontiguousarray(Wq[:, c01] * PRE).astype(bf),
            "wk01": np.ascontiguousarray(Wk[:, c01]).astype(bf),
            "wkq2": np.ascontiguousarray(wkq2).astype(bf),
            "wv": np.ascontiguousarray(Wv[:, cols]).astype(bf),
            "woA": np.ascontiguousarray(Wo[cols, :][0:P]).astype(bf),
            "woB": np.ascontiguousarray(Wo[cols, :][P:HW]).astype(bf),
            "bq01": np.ascontiguousarray((bq[c01] * PRE).reshape(1, 2 * HD)).astype(bf),
            "bq2c": np.ascontiguousarray(bq[c2].reshape(HD, 1)).astype(bf),
            "ident": ident.astype(bf),
        })

    nc = _get_nc()
    res = run_bass_kernel_spmd(nc, in_maps, core_ids=list(range(8)), trace=TRACE)
    LAST_RESULT = res

    out = np.zeros((NSEQ, C), np.float32)
    for c in range(4):
        out[:QB] += res.results[c]["out"].astype(np.float32)
    for c in range(4, 8):
        out[QB:] += res.results[c]["out"].astype(np.float32)
    out += bo + bv @ Wo
    return out.reshape(1, NSEQ, C)


# revision 33
# speedup vs baseline: 1.0270x; 1.0270x over previous
"""Multi-head attention (12 heads, N=4096, C=768) on 8 TRN2 NeuronCores.

Sharding: 8 cores = 4 head-groups x 2 sequence halves.
  core c: heads 3*(c%4) .. 3*(c%4)+2, query rows half (c//4).
Each core computes K/V projections for its 3 heads over the FULL sequence
(inputs are passed with the core's query half rotated to the front, which is
legal because softmax+PV is permutation-invariant along the key axis), Q for
its 2048 query rows, eager attention in S^T orientation (keys on PSUM
partitions), and a partial output projection. Host sums the 4 head-group
partials per sequence half and adds the bias terms.

Performance structure (cost model: matmul time ~ moving free size only):
 - PV runs with exp(S) tiles as the stationary operand ([128k x 128q]) and
   [V | 1] as the 65-wide moving operand, so each accumulation step costs 65
   rows instead of 512. PV accumulates with start=False onto memset-zeroed
   PSUM banks (several accumulation windows share a bank, so the start=True
   bank-wide zero marking cannot be used).
 - Head-2 K and Q projections share one stationary matrix (128 output
   partitions); the missing bq2 is folded into the softmax as a
   per-partition activation bias K.bq2 (constant along q in S^T layout).
 - Q/bq are pre-scaled by 16*log2(e) on the host so the same S feeds both
   the exact-exp path (scale ln2/128) and a future bit-trick exp path.
 - Output projection uses transposed attention tiles as the stationary
   operand: 2 hd-chunks x (512+256) moving columns per 128-query tile.
 - x^T is DMA'd in 8 sequence slabs and the first attention unit is fused
   into the projection loop, so the Act engine (the bottleneck: 192 exp
   instructions) starts within a few microseconds.

Bias algebra (exact): bk drops out of softmax entirely; bv contributes
bv @ Wo to every output row (added on host with bo); bq is folded into Q
(heads 0,1) or into the activation bias (head 2).

All matmuls run with bf16 inputs and fp32 PSUM accumulation.
"""

import math

import numpy as np
import ml_dtypes

import concourse.bass as bass
from concourse import bacc
import concourse.tile as tile
import concourse.mybir as mybir
from concourse.bass_utils import run_bass_kernel_spmd

P = 128
C = 768                    # hidden
NSEQ = 4096                # sequence length
HPC = 3                    # heads per core
HD = 64                    # head dim
HW = HPC * HD              # 192, projection width per core
QB = 2048                  # query rows per core
QBLK = 1024                # query block (PSUM-friendly)
NCH = C // P               # 6 contraction chunks
KT = NSEQ // P             # 32 key tiles
BF16 = mybir.dt.bfloat16
F32 = mybir.dt.float32
AF = mybir.ActivationFunctionType
ALU = mybir.AluOpType

PRE = 16.0 * math.log2(math.e)     # host-side Q pre-scale
SCL = math.log(2.0) / 128.0        # activation scale: exp(S_pre*SCL)=exp(S*0.125)
MAGIC = 16256.0 - 7.5              # bf16 Schraudolph: int16(S_pre + MAGIC) ~ 2^(S_pre/128)
                                   # (-7.5 centers the multiplicative error so
                                   # approximated key-tiles are unbiased vs the
                                   # exact-exp tiles they mix with in softmax)

_CACHE = {}

# set by test.py to capture profiling info
TRACE = False
LAST_RESULT = None


def _build():
    nc = bacc.Bacc("TRN2")

    xT = nc.dram_tensor("xT", [C, NSEQ], BF16, kind="ExternalInput")
    wq01 = nc.dram_tensor("wq01", [C, P], BF16, kind="ExternalInput")
    wk01 = nc.dram_tensor("wk01", [C, P], BF16, kind="ExternalInput")
    wkq2 = nc.dram_tensor("wkq2", [C, P], BF16, kind="ExternalInput")
    wv = nc.dram_tensor("wv", [C, HW], BF16, kind="ExternalInput")
    woA = nc.dram_tensor("woA", [P, C], BF16, kind="ExternalInput")
    woB = nc.dram_tensor("woB", [HD, C], BF16, kind="ExternalInput")
    bq01 = nc.dram_tensor("bq01", [1, P], BF16, kind="ExternalInput")
    bq2c = nc.dram_tensor("bq2c", [HD, 1], BF16, kind="ExternalInput")
    ident = nc.dram_tensor("ident", [P, P], BF16, kind="ExternalInput")
    out = nc.dram_tensor("out", [QB, C], BF16, kind="ExternalOutput")

    NSLAB = 8
    SLAB = NSEQ // NSLAB  # 512

    with tile.TileContext(nc) as tc:
        with (
            tc.tile_pool(name="const", bufs=1) as const,
            tc.tile_pool(name="proj", bufs=1) as proj,
            tc.tile_pool(name="pt", bufs=8) as ptp,
            tc.tile_pool(name="stage", bufs=2) as stage,
            tc.tile_pool(name="psS", bufs=2, space="PSUM") as psS,
            tc.tile_pool(name="psO", bufs=1, space="PSUM") as psO,
            tc.tile_pool(name="psX", bufs=2, space="PSUM") as psX,
        ):
            # ---- load inputs; ordered so Q/K projections can start ASAP ----
            xt = const.tile([P, NCH, NSEQ], BF16)

            def slab_dma(sl):
                nc.sync.dma_start(
                    xt[:, :, sl * SLAB : (sl + 1) * SLAB],
                    xT[:, sl * SLAB : (sl + 1) * SLAB].rearrange(
                        "(c p) n -> p c n", p=P
                    ),
                )

            slab_dma(0)
            wq01_sb = const.tile([P, NCH, P], BF16)
            nc.sync.dma_start(wq01_sb[:], wq01[:].rearrange("(c p) m -> p c m", p=P))
            bq01_sb = const.tile([1, P], BF16)
            nc.sync.dma_start(bq01_sb[:], bq01[:])
            slab_dma(1)
            wk01_sb = const.tile([P, NCH, P], BF16)
            nc.sync.dma_start(wk01_sb[:], wk01[:].rearrange("(c p) m -> p c m", p=P))
            wkq2_sb = const.tile([P, NCH, P], BF16)
            nc.sync.dma_start(wkq2_sb[:], wkq2[:].rearrange("(c p) m -> p c m", p=P))
            wv_sb = const.tile([P, NCH, HW], BF16)
            nc.sync.dma_start(wv_sb[:], wv[:].rearrange("(c p) m -> p c m", p=P))
            for sl in range(2, NSLAB):
                slab_dma(sl)
            ident_sb = const.tile([P, P], BF16)
            nc.sync.dma_start(ident_sb[:], ident[:])
            bq2_sb = const.tile([HD, 1], BF16)
            nc.sync.dma_start(bq2_sb[:], bq2c[:])
            woA_sb = const.tile([P, C], BF16)
            nc.sync.dma_start(woA_sb[:], woA[:])
            woB_sb = const.tile([HD, C], BF16)
            nc.sync.dma_start(woB_sb[:], woB[:])

            ones_row = const.tile([1, 512], BF16)
            nc.vector.memset(ones_row[:], 1.0)

            # ---- persistent projection outputs ----
            KT01 = proj.tile([P, NSEQ], BF16)     # rows 0:64 h0 K^T, 64:128 h1 K^T
            KQ2 = proj.tile([P, NSEQ], BF16)      # rows 0:64 h2 K^T, 64:128 h2 Q^T
            QT01 = proj.tile([P, QB], BF16)       # rows 0:64 h0 Q^T, 64:128 h1 Q^T
            QT2 = proj.tile([HD, QB], BF16)       # h2 Q^T at partition base 0
            V_sb = proj.tile([P, KT, HPC, HD + 1], BF16)  # V + ones column
            O_sb = proj.tile([P, 8, HPC, HD], BF16)       # per-qb normalized attn
            attnTA = proj.tile([P, QB], BF16)     # attn^T rows: h0 d + h1 d
            attnTB = proj.tile([HD, QB], BF16)    # attn^T rows: h2 d
            bias_ln = proj.tile([P, KT], F32)     # (K2.bq2)*0.125 per kt
            bias_pre = proj.tile([P, KT], F32)    # (K2.bq2)*PRE per kt

            nc.vector.memset(V_sb[:, :, :, HD : HD + 1], 1.0)

            # ---------- building blocks ----------
            def q01_proj(nt):
                ps = psX.tile([P, 512], F32, tag="ps")
                for c in range(NCH):
                    nc.tensor.matmul(
                        ps[:], wq01_sb[:, c, :], xt[:, c, nt * 512 : (nt + 1) * 512],
                        start=(c == 0), stop=False,
                    )
                nc.tensor.matmul(
                    ps[:], bq01_sb[0:1, :], ones_row[0:1, :], start=False, stop=True
                )
                nc.vector.tensor_copy(QT01[:, nt * 512 : (nt + 1) * 512], ps[:])

            def k01_proj(nt):
                ps = psX.tile([P, 512], F32, tag="ps")
                for c in range(NCH):
                    nc.tensor.matmul(
                        ps[:], wk01_sb[:, c, :], xt[:, c, nt * 512 : (nt + 1) * 512],
                        start=(c == 0), stop=(c == NCH - 1),
                    )
                nc.vector.tensor_copy(KT01[:, nt * 512 : (nt + 1) * 512], ps[:])

            def kq2_proj(nt):
                ps = psX.tile([P, 512], F32, tag="ps")
                for c in range(NCH):
                    nc.tensor.matmul(
                        ps[:], wkq2_sb[:, c, :], xt[:, c, nt * 512 : (nt + 1) * 512],
                        start=(c == 0), stop=(c == NCH - 1),
                    )
                nc.vector.tensor_copy(KQ2[:, nt * 512 : (nt + 1) * 512], ps[:])
                if nt < QB // 512:
                    nc.vector.tensor_copy(
                        QT2[:, nt * 512 : (nt + 1) * 512], ps[HD:P, :]
                    )

            def v_proj(kt):
                ps = psX.tile([P, 512], F32, tag="ps")
                for c in range(NCH):
                    nc.tensor.matmul(
                        ps[:, 0:HW], xt[:, c, kt * P : (kt + 1) * P], wv_sb[:, c, :],
                        start=(c == 0), stop=(c == NCH - 1),
                    )
                nc.vector.tensor_copy(
                    V_sb[:, kt, :, 0:HD],
                    ps[:, 0:HW].rearrange("p (h d) -> p h d", d=HD),
                )

            def kt_ap(h, kt):
                if h == 0:
                    return KT01[0:HD, kt * P : (kt + 1) * P]
                if h == 1:
                    return KT01[HD:P, kt * P : (kt + 1) * P]
                return KQ2[0:HD, kt * P : (kt + 1) * P]

            def qt_ap(h, qb, qt):
                lo = qb * QBLK + qt * 512
                if h == 0:
                    return QT01[0:HD, lo : lo + 512]
                if h == 1:
                    return QT01[HD:P, lo : lo + 512]
                return QT2[:, lo : lo + 512]

            # attention unit state: two PSUM accumulators (qs 0..3 / 4..7)
            def unit_open():
                oA = psO.tile([P, 4, HD + 1], F32, tag="oA")
                oB = psO.tile([P, 4, HD + 1], F32, tag="oB")
                nc.vector.memset(oA[:], 0.0)
                nc.vector.memset(oB[:], 0.0)
                return oA, oB

            def emit_S(h, qb, kt, eng="A"):
                S_t = psS.tile([P, QBLK], F32, tag="s")
                for qt in range(2):
                    nc.tensor.matmul(
                        S_t[:, qt * 512 : (qt + 1) * 512],
                        kt_ap(h, kt), qt_ap(h, qb, qt),
                        start=True, stop=True,
                    )
                pt = ptp.tile([P, QBLK], BF16, tag="pt")
                if eng == "A":
                    bias = bias_ln[:, kt : kt + 1] if h == 2 else 0.0
                    nc.scalar.activation(pt[:], S_t[:], AF.Exp, bias=bias, scale=SCL)
                else:
                    # bit-trick exp2 on DVE: bf16 bits of 2^(S_pre/128) are
                    # int16(S_pre + MAGIC) (S is pre-scaled by 16*log2 e)
                    pt_i = pt[:].bitcast(mybir.dt.int16)
                    if h == 2:
                        nc.vector.tensor_scalar(
                            pt_i, S_t[:], bias_pre[:, kt : kt + 1], MAGIC,
                            ALU.add, ALU.add,
                        )
                    else:
                        nc.vector.tensor_scalar(pt_i, S_t[:], MAGIC, None, ALU.add)
                return pt

            def emit_PV(h, kt, pt, oA, oB):
                for qs in range(8):
                    o = oA if qs < 4 else oB
                    nc.tensor.matmul(
                        o[:, qs % 4, :],
                        pt[:, qs * P : (qs + 1) * P],
                        V_sb[:, kt, h, :],
                        start=False, stop=False, skip_group_check=True,
                    )

            def unit_close(h, oA, oB):
                # normalize: out = O[:, :64] * (1/den), den in column 64
                recA = stage.tile([P, 4, 1], F32, tag="rec")
                nc.vector.reciprocal(recA[:], oA[:, :, HD : HD + 1])
                recB = stage.tile([P, 4, 1], F32, tag="rec")
                nc.vector.reciprocal(recB[:], oB[:, :, HD : HD + 1])
                for qs in range(8):
                    o, r = (oA, recA) if qs < 4 else (oB, recB)
                    nc.vector.tensor_scalar(
                        O_sb[:, qs, h, :], o[:, qs % 4, 0:HD],
                        r[:, qs % 4, :], None, ALU.mult,
                    )

            def transpose_chunk(qb, qs, pool, tag):
                psT = pool.tile([P, 2 * P], BF16, tag=tag, name="psT")
                nc.tensor.transpose(psT[:, 0:P], O_sb[:, qs, 0:2, :], ident_sb[:])
                nc.tensor.transpose(
                    psT[0:HD, P : 2 * P], O_sb[:, qs, 2, :], ident_sb[:]
                )
                lo = qb * QBLK + qs * P
                nc.vector.tensor_copy(attnTA[:, lo : lo + P], psT[:, 0:P])
                nc.vector.tensor_copy(attnTB[:, lo : lo + P], psT[0:HD, P : 2 * P])

            def outproj_chunk(qb, qs, st, j):
                lo = qb * QBLK + qs * P
                for s0, sw in ((0, 512), (512, 256)):
                    pso = psX.tile([P, 512], F32, tag="ps")
                    nc.tensor.matmul(
                        pso[:, 0:sw], attnTA[:, lo : lo + P],
                        woA_sb[:, s0 : s0 + sw],
                        start=True, stop=False,
                    )
                    nc.tensor.matmul(
                        pso[:, 0:sw], attnTB[0:HD, lo : lo + P],
                        woB_sb[:, s0 : s0 + sw],
                        start=False, stop=True,
                    )
                    nc.vector.tensor_copy(st[:, j, s0 : s0 + sw], pso[:, 0:sw])

            def qb_finish_chunk(qb, qs):
                # transpose one 128-query tile of normalized attn into attn^T,
                # then output-project and DMA it out
                transpose_chunk(qb, qs, psX, "ps")
                st = stage.tile([P, 1, C], BF16, tag="st")
                outproj_chunk(qb, qs, st, 0)
                lo = qb * QBLK + qs * P
                nc.sync.dma_start(out[lo : lo + P, :], st[:, 0, :])

            def qb_finish_tail(qb):
                # pipelined epilogue for the final query block: all transposes
                # first (PSUM accumulators are free by now), then paired
                # output-projection chunks sharing one DMA each
                for qs in range(8):
                    transpose_chunk(qb, qs, psO, "oA" if qs % 2 == 0 else "oB")
                for pair in range(4):
                    st = stage.tile([P, 2, C], BF16, tag="st2")
                    for j in (0, 1):
                        outproj_chunk(qb, 2 * pair + j, st, j)
                    lo = qb * QBLK + pair * 2 * P
                    nc.sync.dma_start(
                        out[lo : lo + 2 * P, :].rearrange("(j p) c -> p j c", p=P),
                        st[:],
                    )

            def kbq2_bias():
                psB = psX.tile([P, KT], F32, tag="ps")
                for kt in range(KT):
                    nc.tensor.matmul(
                        psB[:, kt : kt + 1], kt_ap(2, kt), bq2_sb[:],
                        start=True, stop=True,
                    )
                nc.vector.tensor_scalar(bias_ln[:], psB[:], 0.125, None, ALU.mult)
                nc.vector.tensor_scalar(bias_pre[:], psB[:], PRE, None, ALU.mult)

            # ---------- schedule ----------
            # One flat software-pipelined stream over 192 (unit, kt) steps.
            # Step i emits S+exp for step i, then PV for step i-2 (the lag
            # gives the DVE time to close/zero PSUM accumulators at unit
            # boundaries without stalling the PE pipeline). Background work
            # (leftover projections, h2 bias, qb0 epilogue) is interleaved
            # one small chunk every other step to stay under the Act step
            # budget (~1038ns).
            LAG = 2
            NST = 6 * KT
            UNITS = [(h, qb) for qb in (0, 1) for h in (0, 1, 2)]

            bg = []
            bg.append(lambda: q01_proj(2))
            bg.append(lambda: q01_proj(3))
            bg.append(kbq2_bias)
            for qs_ in range(8):
                bg.append(lambda qs=qs_: qb_finish_chunk(0, qs))

            pts = {}   # step -> pt tile
            os_ = {}   # unit -> (oA, oB)

            def emit_PV_step(j):
                h, qb = UNITS[j // KT]
                kt = j % KT
                if kt == 0:
                    os_[j // KT] = unit_open()
                emit_PV(h, kt, pts.pop(j), *os_[j // KT])
                if kt == KT - 1:
                    unit_close(h, *os_.pop(j // KT))

            # per-step exp engine: Act / DVE balanced by cost-model throughput
            PROPAT = "AD"      # prologue: PE-bound, DVE has slack
            STEADYPAT = "AADAAADAAD"

            def exp_eng(i):
                return PROPAT[i % 2] if i < KT else STEADYPAT[i % 10]

            def stream_step(i, xa=None):
                h, qb = UNITS[i // KT]
                pts[i] = emit_S(h, qb, i % KT, exp_eng(i))
                if i >= LAG:
                    emit_PV_step(i - LAG)
                # background chunk (only after the fused projection prologue;
                # qb0 epilogue chunks additionally wait for (qb0,h2) close)
                if i >= KT and (i % 3 == 0) and bg:
                    if not (len(bg) <= 8 and i < 3 * KT + LAG + 2):
                        bg.pop(0)()

            # fused prologue: projections + unit 0, pipelined.  S/exp for a
            # block go out right after its K tile so the Act engine starts
            # early; V lags behind (PV only needs it LAG steps later).
            q01_proj(0)
            q01_proj(1)
            for nt in range(NSLAB):
                k01_proj(nt)
                kq2_proj(nt)
                for kt in range(4 * nt, 4 * nt + 4):
                    v_proj(kt)
                for kt in range(4 * nt, 4 * nt + 4):
                    stream_step(kt)
            for i in range(KT, NST):
                stream_step(i)
            # drain: last PVs + closes + leftovers + qb1 epilogue
            for j in range(NST - LAG, NST):
                emit_PV_step(j)
            while bg:
                bg.pop(0)()
            qb_finish_tail(1)

    if hasattr(nc, "compile"):
        nc.compile()
    return nc


def _get_nc():
    if "nc" not in _CACHE:
        _CACHE["nc"] = _build()
    return _CACHE["nc"]


def kernel(x, Wq, bq, Wk, bk, Wv, bv, Wo, bo):
    global LAST_RESULT
    x = np.asarray(x, dtype=np.float32)
    Wq = np.asarray(Wq, dtype=np.float32)
    Wk = np.asarray(Wk, dtype=np.float32)
    Wv = np.asarray(Wv, dtype=np.float32)
    Wo = np.asarray(Wo, dtype=np.float32)
    bq = np.asarray(bq, dtype=np.float32)
    bv = np.asarray(bv, dtype=np.float32)
    bo = np.asarray(bo, dtype=np.float32)

    B, N, Ch = x.shape
    assert (B, N, Ch) == (1, NSEQ, C)
    xT_full = np.ascontiguousarray(x[0].T)  # [C, N] f32

    bf = ml_dtypes.bfloat16
    ident = np.eye(P, dtype=np.float32)
    in_maps = []
    for c in range(8):
        qhalf = c // 4
        hbase = HPC * (c % 4)
        cols = slice(hbase * HD, hbase * HD + HW)
        c01 = slice(hbase * HD, hbase * HD + 2 * HD)
        c2 = slice(hbase * HD + 2 * HD, hbase * HD + HW)
        if qhalf == 0:
            xTc = xT_full
        else:
            xTc = np.concatenate([xT_full[:, QB:], xT_full[:, :QB]], axis=1)
        wkq2 = np.concatenate([Wk[:, c2], Wq[:, c2] * PRE], axis=1)
        in_maps.append({
            "xT": np.ascontiguousarray(xTc).astype(bf),
            "wq01": np.ascontiguousarray(Wq[:, c01] * PRE).astype(bf),
            "wk01": np.ascontiguousarray(Wk[:, c01]).astype(bf),
            "wkq2": np.ascontiguousarray(wkq2).astype(bf),
            "wv": np.ascontiguousarray(Wv[:, cols]).astype(bf),
            "woA": np.ascontiguousarray(Wo[cols, :][0:P]).astype(bf),
            "woB": np.ascontiguousarray(Wo[cols, :][P:HW]).astype(bf),
            "bq01": np.ascontiguousarray((bq[c01] * PRE).reshape(1, 2 * HD)).astype(bf),
            "bq2c": np.ascontiguousarray(bq[c2].reshape(HD, 1)).astype(bf),
            "ident": ident.astype(bf),
        })

    nc = _get_nc()
    res = run_bass_kernel_spmd(nc, in_maps, core_ids=list(range(8)), trace=TRACE)
    LAST_RESULT = res

    out = np.zeros((NSEQ, C), np.float32)
    for c in range(4):
        out[:QB] += res.results[c]["out"].astype(np.float32)
    for c in range(4, 8):
        out[QB:] += res.results[c]["out"].astype(np.float32)
    out += bo + bv @ Wo
    return out.reshape(1, NSEQ, C)


# revision 34
# speedup vs baseline: 1.0289x; 1.0019x over previous
"""Multi-head attention (12 heads, N=4096, C=768) on 8 TRN2 NeuronCores.

Sharding: 8 cores = 4 head-groups x 2 sequence halves.
  core c: heads 3*(c%4) .. 3*(c%4)+2, query rows half (c//4).
Each core computes K/V projections for its 3 heads over the FULL sequence
(inputs are passed with the core's query half rotated to the front, which is
legal because softmax+PV is permutation-invariant along the key axis), Q for
its 2048 query rows, eager attention in S^T orientation (keys on PSUM
partitions), and a partial output projection. Host sums the 4 head-group
partials per sequence half and adds the bias terms.

Performance structure (cost model: matmul time ~ moving free size only):
 - PV runs with exp(S) tiles as the stationary operand ([128k x 128q]) and
   [V | 1] as the 65-wide moving operand, so each accumulation step costs 65
   rows instead of 512. PV accumulates with start=False onto memset-zeroed
   PSUM banks (several accumulation windows share a bank, so the start=True
   bank-wide zero marking cannot be used).
 - Head-2 K and Q projections share one stationary matrix (128 output
   partitions); the missing bq2 is folded into the softmax as a
   per-partition activation bias K.bq2 (constant along q in S^T layout).
 - Q/bq are pre-scaled by 16*log2(e) on the host so the same S feeds both
   the exact-exp path (scale ln2/128) and a future bit-trick exp path.
 - Output projection uses transposed attention tiles as the stationary
   operand: 2 hd-chunks x (512+256) moving columns per 128-query tile.
 - x^T is DMA'd in 8 sequence slabs and the first attention unit is fused
   into the projection loop, so the Act engine (the bottleneck: 192 exp
   instructions) starts within a few microseconds.

Bias algebra (exact): bk drops out of softmax entirely; bv contributes
bv @ Wo to every output row (added on host with bo); bq is folded into Q
(heads 0,1) or into the activation bias (head 2).

All matmuls run with bf16 inputs and fp32 PSUM accumulation.
"""

import math

import numpy as np
import ml_dtypes

import concourse.bass as bass
from concourse import bacc
import concourse.tile as tile
import concourse.mybir as mybir
from concourse.bass_utils import run_bass_kernel_spmd

P = 128
C = 768                    # hidden
NSEQ = 4096                # sequence length
HPC = 3                    # heads per core
HD = 64                    # head dim
HW = HPC * HD              # 192, projection width per core
QB = 2048                  # query rows per core
QBLK = 1024                # query block (PSUM-friendly)
NCH = C // P               # 6 contraction chunks
KT = NSEQ // P             # 32 key tiles
BF16 = mybir.dt.bfloat16
F32 = mybir.dt.float32
AF = mybir.ActivationFunctionType
ALU = mybir.AluOpType

PRE = 16.0 * math.log2(math.e)     # host-side Q pre-scale
SCL = math.log(2.0) / 128.0        # activation scale: exp(S_pre*SCL)=exp(S*0.125)
MAGIC = 16256.0 - 7.5              # bf16 Schraudolph: int16(S_pre + MAGIC) ~ 2^(S_pre/128)
                                   # (-7.5 centers the multiplicative error so
                                   # approximated key-tiles are unbiased vs the
                                   # exact-exp tiles they mix with in softmax)

_CACHE = {}

# set by test.py to capture profiling info
TRACE = False
LAST_RESULT = None


def _build():
    nc = bacc.Bacc("TRN2")

    xT = nc.dram_tensor("xT", [C, NSEQ], BF16, kind="ExternalInput")
    wq01 = nc.dram_tensor("wq01", [C, P], BF16, kind="ExternalInput")
    wk01 = nc.dram_tensor("wk01", [C, P], BF16, kind="ExternalInput")
    wkq2 = nc.dram_tensor("wkq2", [C, P], BF16, kind="ExternalInput")
    wv = nc.dram_tensor("wv", [C, HW], BF16, kind="ExternalInput")
    woA = nc.dram_tensor("woA", [P, C], BF16, kind="ExternalInput")
    woB = nc.dram_tensor("woB", [HD, C], BF16, kind="ExternalInput")
    bq01 = nc.dram_tensor("bq01", [1, P], BF16, kind="ExternalInput")
    bq2c = nc.dram_tensor("bq2c", [HD, 1], BF16, kind="ExternalInput")
    ident = nc.dram_tensor("ident", [P, P], BF16, kind="ExternalInput")
    out = nc.dram_tensor("out", [QB, C], BF16, kind="ExternalOutput")

    NSLAB = 8
    SLAB = NSEQ // NSLAB  # 512

    with tile.TileContext(nc) as tc:
        with (
            tc.tile_pool(name="const", bufs=1) as const,
            tc.tile_pool(name="proj", bufs=1) as proj,
            tc.tile_pool(name="pt", bufs=8) as ptp,
            tc.tile_pool(name="stage", bufs=2) as stage,
            tc.tile_pool(name="psS", bufs=2, space="PSUM") as psS,
            tc.tile_pool(name="psO", bufs=1, space="PSUM") as psO,
            tc.tile_pool(name="psX", bufs=2, space="PSUM") as psX,
        ):
            # ---- load inputs; ordered so Q/K projections can start ASAP ----
            xt = const.tile([P, NCH, NSEQ], BF16)

            def slab_dma(sl):
                nc.sync.dma_start(
                    xt[:, :, sl * SLAB : (sl + 1) * SLAB],
                    xT[:, sl * SLAB : (sl + 1) * SLAB].rearrange(
                        "(c p) n -> p c n", p=P
                    ),
                )

            slab_dma(0)
            wq01_sb = const.tile([P, NCH, P], BF16)
            nc.sync.dma_start(wq01_sb[:], wq01[:].rearrange("(c p) m -> p c m", p=P))
            bq01_sb = const.tile([1, P], BF16)
            nc.sync.dma_start(bq01_sb[:], bq01[:])
            slab_dma(1)
            wk01_sb = const.tile([P, NCH, P], BF16)
            nc.sync.dma_start(wk01_sb[:], wk01[:].rearrange("(c p) m -> p c m", p=P))
            wkq2_sb = const.tile([P, NCH, P], BF16)
            nc.sync.dma_start(wkq2_sb[:], wkq2[:].rearrange("(c p) m -> p c m", p=P))
            wv_sb = const.tile([P, NCH, HW], BF16)
            nc.sync.dma_start(wv_sb[:], wv[:].rearrange("(c p) m -> p c m", p=P))
            for sl in range(2, NSLAB):
                slab_dma(sl)
            ident_sb = const.tile([P, P], BF16)
            nc.sync.dma_start(ident_sb[:], ident[:])
            bq2_sb = const.tile([HD, 1], BF16)
            nc.sync.dma_start(bq2_sb[:], bq2c[:])
            woA_sb = const.tile([P, C], BF16)
            nc.sync.dma_start(woA_sb[:], woA[:])
            woB_sb = const.tile([HD, C], BF16)
            nc.sync.dma_start(woB_sb[:], woB[:])

            ones_row = const.tile([1, 512], BF16)
            nc.vector.memset(ones_row[:], 1.0)

            # ---- persistent projection outputs ----
            KT01 = proj.tile([P, NSEQ], BF16)     # rows 0:64 h0 K^T, 64:128 h1 K^T
            KQ2 = proj.tile([P, NSEQ], BF16)      # rows 0:64 h2 K^T, 64:128 h2 Q^T
            QT01 = proj.tile([P, QB], BF16)       # rows 0:64 h0 Q^T, 64:128 h1 Q^T
            QT2 = proj.tile([HD, QB], BF16)       # h2 Q^T at partition base 0
            V_sb = proj.tile([P, KT, HPC, HD + 1], BF16)  # V + ones column
            O_sb = proj.tile([P, 8, HPC, HD], BF16)       # per-qb normalized attn
            attnTA = proj.tile([P, QB], BF16)     # attn^T rows: h0 d + h1 d
            attnTB = proj.tile([HD, QB], BF16)    # attn^T rows: h2 d
            bias_ln = proj.tile([P, KT], F32)     # (K2.bq2)*0.125 per kt
            bias_pre = proj.tile([P, KT], F32)    # (K2.bq2)*PRE per kt

            nc.vector.memset(V_sb[:, :, :, HD : HD + 1], 1.0)

            # ---------- building blocks ----------
            def q01_proj(nt):
                ps = psX.tile([P, 512], F32, tag="ps")
                for c in range(NCH):
                    nc.tensor.matmul(
                        ps[:], wq01_sb[:, c, :], xt[:, c, nt * 512 : (nt + 1) * 512],
                        start=(c == 0), stop=False,
                    )
                nc.tensor.matmul(
                    ps[:], bq01_sb[0:1, :], ones_row[0:1, :], start=False, stop=True
                )
                nc.vector.tensor_copy(QT01[:, nt * 512 : (nt + 1) * 512], ps[:])

            def k01_proj(nt):
                ps = psX.tile([P, 512], F32, tag="ps")
                for c in range(NCH):
                    nc.tensor.matmul(
                        ps[:], wk01_sb[:, c, :], xt[:, c, nt * 512 : (nt + 1) * 512],
                        start=(c == 0), stop=(c == NCH - 1),
                    )
                nc.vector.tensor_copy(KT01[:, nt * 512 : (nt + 1) * 512], ps[:])

            def kq2_proj(nt):
                ps = psX.tile([P, 512], F32, tag="ps")
                for c in range(NCH):
                    nc.tensor.matmul(
                        ps[:], wkq2_sb[:, c, :], xt[:, c, nt * 512 : (nt + 1) * 512],
                        start=(c == 0), stop=(c == NCH - 1),
                    )
                nc.vector.tensor_copy(KQ2[:, nt * 512 : (nt + 1) * 512], ps[:])
                if nt < QB // 512:
                    nc.vector.tensor_copy(
                        QT2[:, nt * 512 : (nt + 1) * 512], ps[HD:P, :]
                    )

            def v_proj(kt):
                ps = psX.tile([P, 512], F32, tag="ps")
                for c in range(NCH):
                    nc.tensor.matmul(
                        ps[:, 0:HW], xt[:, c, kt * P : (kt + 1) * P], wv_sb[:, c, :],
                        start=(c == 0), stop=(c == NCH - 1),
                    )
                nc.vector.tensor_copy(
                    V_sb[:, kt, :, 0:HD],
                    ps[:, 0:HW].rearrange("p (h d) -> p h d", d=HD),
                )

            def kt_ap(h, kt):
                if h == 0:
                    return KT01[0:HD, kt * P : (kt + 1) * P]
                if h == 1:
                    return KT01[HD:P, kt * P : (kt + 1) * P]
                return KQ2[0:HD, kt * P : (kt + 1) * P]

            def qt_ap(h, qb, qt):
                lo = qb * QBLK + qt * 512
                if h == 0:
                    return QT01[0:HD, lo : lo + 512]
                if h == 1:
                    return QT01[HD:P, lo : lo + 512]
                return QT2[:, lo : lo + 512]

            # attention unit state: two PSUM accumulators (qs 0..3 / 4..7)
            def unit_open():
                oA = psO.tile([P, 4, HD + 1], F32, tag="oA")
                oB = psO.tile([P, 4, HD + 1], F32, tag="oB")
                nc.vector.memset(oA[:], 0.0)
                nc.vector.memset(oB[:], 0.0)
                return oA, oB

            def emit_S(h, qb, kt, eng="A"):
                S_t = psS.tile([P, QBLK], F32, tag="s")
                for qt in range(2):
                    nc.tensor.matmul(
                        S_t[:, qt * 512 : (qt + 1) * 512],
                        kt_ap(h, kt), qt_ap(h, qb, qt),
                        start=True, stop=True,
                    )
                pt = ptp.tile([P, QBLK], BF16, tag="pt")
                if eng == "A":
                    bias = bias_ln[:, kt : kt + 1] if h == 2 else 0.0
                    nc.scalar.activation(pt[:], S_t[:], AF.Exp, bias=bias, scale=SCL)
                else:
                    # bit-trick exp2 on DVE: bf16 bits of 2^(S_pre/128) are
                    # int16(S_pre + MAGIC) (S is pre-scaled by 16*log2 e)
                    pt_i = pt[:].bitcast(mybir.dt.int16)
                    if h == 2:
                        nc.vector.tensor_scalar(
                            pt_i, S_t[:], bias_pre[:, kt : kt + 1], MAGIC,
                            ALU.add, ALU.add,
                        )
                    else:
                        nc.vector.tensor_scalar(pt_i, S_t[:], MAGIC, None, ALU.add)
                return pt

            def emit_PV(h, kt, pt, oA, oB):
                for qs in range(8):
                    o = oA if qs < 4 else oB
                    nc.tensor.matmul(
                        o[:, qs % 4, :],
                        pt[:, qs * P : (qs + 1) * P],
                        V_sb[:, kt, h, :],
                        start=False, stop=False, skip_group_check=True,
                    )

            def unit_close(h, oA, oB):
                # normalize: out = O[:, :64] * (1/den), den in column 64
                recA = stage.tile([P, 4, 1], F32, tag="rec")
                nc.vector.reciprocal(recA[:], oA[:, :, HD : HD + 1])
                recB = stage.tile([P, 4, 1], F32, tag="rec")
                nc.vector.reciprocal(recB[:], oB[:, :, HD : HD + 1])
                for qs in range(8):
                    o, r = (oA, recA) if qs < 4 else (oB, recB)
                    nc.vector.tensor_scalar(
                        O_sb[:, qs, h, :], o[:, qs % 4, 0:HD],
                        r[:, qs % 4, :], None, ALU.mult,
                    )

            def transpose_chunk(qb, qs, pool, tag):
                psT = pool.tile([P, 2 * P], BF16, tag=tag, name="psT")
                nc.tensor.transpose(psT[:, 0:P], O_sb[:, qs, 0:2, :], ident_sb[:])
                nc.tensor.transpose(
                    psT[0:HD, P : 2 * P], O_sb[:, qs, 2, :], ident_sb[:]
                )
                lo = qb * QBLK + qs * P
                nc.vector.tensor_copy(attnTA[:, lo : lo + P], psT[:, 0:P])
                nc.vector.tensor_copy(attnTB[:, lo : lo + P], psT[0:HD, P : 2 * P])

            def outproj_chunk(qb, qs, st, j):
                lo = qb * QBLK + qs * P
                for s0, sw in ((0, 512), (512, 256)):
                    pso = psX.tile([P, 512], F32, tag="ps")
                    nc.tensor.matmul(
                        pso[:, 0:sw], attnTA[:, lo : lo + P],
                        woA_sb[:, s0 : s0 + sw],
                        start=True, stop=False,
                    )
                    nc.tensor.matmul(
                        pso[:, 0:sw], attnTB[0:HD, lo : lo + P],
                        woB_sb[:, s0 : s0 + sw],
                        start=False, stop=True,
                    )
                    nc.vector.tensor_copy(st[:, j, s0 : s0 + sw], pso[:, 0:sw])

            def qb_finish_chunk(qb, qs):
                # transpose one 128-query tile of normalized attn into attn^T,
                # then output-project and DMA it out
                transpose_chunk(qb, qs, psX, "ps")
                st = stage.tile([P, 1, C], BF16, tag="st")
                outproj_chunk(qb, qs, st, 0)
                lo = qb * QBLK + qs * P
                nc.sync.dma_start(out[lo : lo + P, :], st[:, 0, :])

            def qb_finish_tail(qb):
                # pipelined epilogue for the final query block: all transposes
                # first (PSUM accumulators are free by now), then paired
                # output-projection chunks sharing one DMA each
                for qs in range(8):
                    transpose_chunk(qb, qs, psO, "oA" if qs % 2 == 0 else "oB")
                for pair in range(4):
                    st = stage.tile([P, 2, C], BF16, tag="st2")
                    for j in (0, 1):
                        outproj_chunk(qb, 2 * pair + j, st, j)
                    lo = qb * QBLK + pair * 2 * P
                    nc.sync.dma_start(
                        out[lo : lo + 2 * P, :].rearrange("(j p) c -> p j c", p=P),
                        st[:],
                    )

            def kbq2_bias():
                psB = psX.tile([P, KT], F32, tag="ps")
                for kt in range(KT):
                    nc.tensor.matmul(
                        psB[:, kt : kt + 1], kt_ap(2, kt), bq2_sb[:],
                        start=True, stop=True,
                    )
                nc.vector.tensor_scalar(bias_ln[:], psB[:], 0.125, None, ALU.mult)
                nc.vector.tensor_scalar(bias_pre[:], psB[:], PRE, None, ALU.mult)

            # ---------- schedule ----------
            # One flat software-pipelined stream over 192 (unit, kt) steps.
            # Step i emits S+exp for step i, then PV for step i-2 (the lag
            # gives the DVE time to close/zero PSUM accumulators at unit
            # boundaries without stalling the PE pipeline). Background work
            # (leftover projections, h2 bias, qb0 epilogue) is interleaved
            # one small chunk every other step to stay under the Act step
            # budget (~1038ns).
            LAG = 2
            NST = 6 * KT
            UNITS = [(h, qb) for qb in (0, 1) for h in (0, 1, 2)]

            bg = []
            bg.append(lambda: q01_proj(2))
            bg.append(lambda: q01_proj(3))
            bg.append(kbq2_bias)
            for qs_ in range(8):
                bg.append(lambda qs=qs_: qb_finish_chunk(0, qs))

            pts = {}   # step -> pt tile
            os_ = {}   # unit -> (oA, oB)

            def emit_PV_step(j):
                h, qb = UNITS[j // KT]
                kt = j % KT
                if kt == 0:
                    os_[j // KT] = unit_open()
                emit_PV(h, kt, pts.pop(j), *os_[j // KT])
                if kt == KT - 1:
                    unit_close(h, *os_.pop(j // KT))

            # per-step exp engine: Act / DVE balanced by cost-model throughput
            PROPAT = "AD"      # prologue: PE-bound, DVE has slack
            STEADYPAT = "AADAADAADAADAAD"

            def exp_eng(i):
                return PROPAT[i % 2] if i < KT else STEADYPAT[i % 10]

            def stream_step(i, xa=None):
                h, qb = UNITS[i // KT]
                pts[i] = emit_S(h, qb, i % KT, exp_eng(i))
                if i >= LAG:
                    emit_PV_step(i - LAG)
                # background chunk (only after the fused projection prologue;
                # qb0 epilogue chunks additionally wait for (qb0,h2) close)
                if i >= KT and (i % 3 == 0) and bg:
                    if not (len(bg) <= 8 and i < 3 * KT + LAG + 2):
                        bg.pop(0)()

            # fused prologue: projections + unit 0, pipelined.  S/exp for a
            # block go out right after its K tile so the Act engine starts
            # early; V lags behind (PV only needs it LAG steps later).
            q01_proj(0)
            q01_proj(1)
            for nt in range(NSLAB):
                k01_proj(nt)
                kq2_proj(nt)
                for kt in range(4 * nt, 4 * nt + 4):
                    v_proj(kt)
                for kt in range(4 * nt, 4 * nt + 4):
                    stream_step(kt)
            for i in range(KT, NST):
                stream_step(i)
            # drain: last PVs + closes + leftovers + qb1 epilogue
            for j in range(NST - LAG, NST):
                emit_PV_step(j)
            while bg:
                bg.pop(0)()
            qb_finish_tail(1)

    if hasattr(nc, "compile"):
        nc.compile()
    return nc


def _get_nc():
    if "nc" not in _CACHE:
        _CACHE["nc"] = _build()
    return _CACHE["nc"]


def kernel(x, Wq, bq, Wk, bk, Wv, bv, Wo, bo):
    global LAST_RESULT
    x = np.asarray(x, dtype=np.float32)
    Wq = np.asarray(Wq, dtype=np.float32)
    Wk = np.asarray(Wk, dtype=np.float32)
    Wv = np.asarray(Wv, dtype=np.float32)
    Wo = np.asarray(Wo, dtype=np.float32)
    bq = np.asarray(bq, dtype=np.float32)
    bv = np.asarray(bv, dtype=np.float32)
    bo = np.asarray(bo, dtype=np.float32)

    B, N, Ch = x.shape
    assert (B, N, Ch) == (1, NSEQ, C)
    xT_full = np.ascontiguousarray(x[0].T)  # [C, N] f32

    bf = ml_dtypes.bfloat16
    ident = np.eye(P, dtype=np.float32)
    in_maps = []
    for c in range(8):
        qhalf = c // 4
        hbase = HPC * (c % 4)
        cols = slice(hbase * HD, hbase * HD + HW)
        c01 = slice(hbase * HD, hbase * HD + 2 * HD)
        c2 = slice(hbase * HD + 2 * HD, hbase * HD + HW)
        if qhalf == 0:
            xTc = xT_full
        else:
            xTc = np.concatenate([xT_full[:, QB:], xT_full[:, :QB]], axis=1)
        wkq2 = np.concatenate([Wk[:, c2], Wq[:, c2] * PRE], axis=1)
        in_maps.append({
            "xT": np.ascontiguousarray(xTc).astype(bf),
            "wq01": np.ascontiguousarray(Wq[:, c01] * PRE).astype(bf),
            "wk01": np.ascontiguousarray(Wk[:, c01]).astype(bf),
            "wkq2": np.ascontiguousarray(wkq2).astype(bf),
            "wv": np.ascontiguousarray(Wv[:, cols]).astype(bf),
            "woA": np.ascontiguousarray(Wo[cols, :][0:P]).astype(bf),
            "woB": np.ascontiguousarray(Wo[cols, :][P:HW]).astype(bf),
            "bq01": np.ascontiguousarray((bq[c01] * PRE).reshape(1, 2 * HD)).astype(bf),
            "bq2c": np.ascontiguousarray(bq[c2].reshape(HD, 1)).astype(bf),
            "ident": ident.astype(bf),
        })

    nc = _get_nc()
    res = run_bass_kernel_spmd(nc, in_maps, core_ids=list(range(8)), trace=TRACE)
    LAST_RESULT = res

    out = np.zeros((NSEQ, C), np.float32)
    for c in range(4):
        out[:QB] += res.results[c]["out"].astype(np.float32)
    for c in range(4, 8):
        out[QB:] += res.results[c]["out"].astype(np.float32)
    out += bo + bv @ Wo
    return out.reshape(1, NSEQ, C)
